# revision 5
# baseline (speedup 1.0000x reference)
"""Trainium2 Bass kernel for the DLGN kernel-machine problem.

Reference computation (fp32):
    ig = inp @ gating[0]; dg = data @ gating[0]
    K  = sig(B*ig) @ sig(B*dg).T
    for l in 1..3:
        ig = ig @ gating[l]; dg = dg @ gating[l]
        K *= (sig(B*ig) @ sig(B*dg).T) / 512
    out = K @ alphas                      # [n_inp]

Strategy (8 NeuronCores, 2x4 shard as before), v2 rewrite:
  - FLATTENED GATE CHAIN: host precomputes cumulative weight products
    W~_l = W_1...W_l (fp64), so layer l's pre-activation is x0 @ W~_l
    directly from the original input. This removes the sequential
    layer dependency and all PSUM->SBUF chain copies (was ~63us DVE).
  - fp8 DoubleRow gate matmuls with hi/lo split operands: x ~ xh + xl
    and W~ ~ Wh + Wl (e4m3 pairs, ~14-bit effective). Three-term
    product (xh@Wh + xl@Wh + xh@Wl, lo@lo dropped) keeps the final
    metric at ~1.5e-2 (measured in numpy emulation; gate is 2e-2)
    while running the gates at fp8-DR speed: 2x fewer PE cycles than
    bf16 even with 3 terms.
  - Asymmetric centering (as before): i-side sigmoid s, d-side
    t = tanh(2x); 2K_l = Si_l + s8.t8 with Si = rowsum(s8) EXACT via
    fp8 matmul vs ones. All matmuls fp8-DR: no dtype mode switches.
  - Combine, one fixed schedule per (stripe, ic) tile:
      DVE  stt: kblk = (kps0 + S0) * alphas      (PSUM read)
      DVE  stt: kblk = (kps1 + S1) * kblk        (PSUM read)
      ACT  copy: u23 = kps2|kps3 [128,2,512]     (paired 2-bank PSUM read)
      Pool stt: kblk = (u23[0] + S2) * kblk      (SBUF, gpsimd)
      DVE  stt: (u23[1] + S3) * kblk, accum_out -> parts[s,ic]
    Engine busy est/core: PE ~137us, ACT ~134, DVE ~118, Pool ~45.
  - PSUM: phase 1 (i-gates): pair pool [128,2,512]x3 + Si bank = 7.
    phase 2: shared pool tags AB (d-gates + K L0|L1) 2x2 banks and
    B (K L2|L3) 2x2 banks = 8.
"""

import numpy as np

import concourse.tile as tile
from concourse import bacc, mybir
from concourse.bass_utils import run_bass_kernel_spmd

BETA = 4.0
WIDTH = 512
DEPTH = 4
DIM = 512
N_I = 4096
N_D = 8192
R, C = 2, 4
NI_SH = N_I // R  # 2048
ND_SH = N_D // C  # 2048
D_STRIPE = 512
N_STRIPES = ND_SH // D_STRIPE  # 4
I_CHUNKS = NI_SH // 128  # 16
NB = NI_SH // 512  # 4 column blocks on the i side
C_SCALE = (0.5**DEPTH) / float(WIDTH ** (DEPTH - 1))  # 2^-31, exact
WSC = 32.0  # weight prescale for fp8
XSC = 16.0  # input prescale for fp8
SIG_SCALE = BETA / (WSC * XSC)  # sig(4x) = sig(psum * 2^-7)
TANH_SCALE = (BETA / 2) / (WSC * XSC)  # tanh(2x) = tanh(psum * 2^-8)

F32 = mybir.dt.float32
FP8 = mybir.dt.float8e4
AFT = mybir.ActivationFunctionType
MULT = mybir.AluOpType.mult
ADD = mybir.AluOpType.add
DRM = mybir.MatmulPerfMode.DoubleRow

_NC = None

# (x-variant, w-variant) matmul terms: hi@hi + lo@hi + hi@lo
HILO_TERMS = [(0, 0), (1, 0), (0, 1)]


def _gate_pair(nc, pool, tag, wq, x8, out8, l, mp, cols, aft, scale):
    """One [128, 2, 512] gate pair tile: 12 fp8-DR matmuls + 1 activation.

    mp is the m-chunk pair (m = 2*mp + m2); cols is the 512-wide column
    slice of the i/d axis being produced."""
    gt = pool.tile([128, 2, 512], F32, tag=tag)
    for m2 in range(2):
        m = 2 * mp + m2
        n = 0
        for h in range(2):
            for xv, wv in HILO_TERMS:
                nc.tensor.matmul(
                    gt[:, m2, :],
                    wq[:, l, wv, h, :, m * 128 : (m + 1) * 128],
                    x8[:, xv, h, :, cols],
                    start=(n == 0),
                    stop=(n == 5),
                    perf_mode=DRM,
                )
                n += 1
    nc.scalar.activation(out8[:, l, 2 * mp : 2 * mp + 2, cols], gt[:], aft, scale=scale)


def _build(repeat=1):
    nc = bacc.Bacc("TRN2", target_bir_lowering=False, debug=False, num_devices=8)

    wq_d = nc.dram_tensor("wq", [128, DEPTH, 2, 2, 2, DIM], FP8, kind="ExternalInput")
    xi_d = nc.dram_tensor("xi", [128, 2, 2, 2, NI_SH], FP8, kind="ExternalInput")
    xd_d = nc.dram_tensor("xd", [128, 2, 2, 2, ND_SH], FP8, kind="ExternalInput")
    alphas_d = nc.dram_tensor("alphas_s", [128, ND_SH], F32, kind="ExternalInput")
    y_d = nc.dram_tensor("y", [128, I_CHUNKS], F32, kind="ExternalOutput")

    from contextlib import nullcontext

    with tile.TileContext(nc) as tc:
        with (
            tc.tile_pool(name="w", bufs=1) as wpool,
            tc.tile_pool(name="x", bufs=1) as xpool,
            tc.tile_pool(name="sig", bufs=1) as sigpool,
            tc.tile_pool(name="misc", bufs=1) as mpool,
            tc.tile_pool(name="kblk", bufs=4) as kpool,
            tc.tile_pool(name="u", bufs=6) as upool,
            tc.For_i(0, repeat, 1) if repeat > 1 else nullcontext(),
        ):
            wq = wpool.tile([128, DEPTH, 2, 2, 2, DIM], FP8)
            for l in range(DEPTH):
                nc.sync.dma_start(wq[:, l], wq_d.ap()[:, l])
            xi = xpool.tile([128, 2, 2, 2, NI_SH], FP8, name="xi")
            for nb in range(NB):
                sl = slice(nb * 512, (nb + 1) * 512)
                nc.sync.dma_start(xi[:, :, :, :, sl], xi_d.ap()[:, :, :, :, sl])
            xd = xpool.tile([128, 2, 2, 2, ND_SH], FP8, name="xd")
            for s in range(N_STRIPES):
                sl = slice(s * D_STRIPE, (s + 1) * D_STRIPE)
                nc.sync.dma_start(xd[:, :, :, :, sl], xd_d.ap()[:, :, :, :, sl])
            alp = mpool.tile([128, ND_SH], F32, name="alp")
            nc.sync.dma_start(alp[:], alphas_d.ap())

            ones8_t = mpool.tile([128, 2, 16], FP8, name="ones8")
            nc.gpsimd.memset(ones8_t[:], 1.0)
            ones8 = ones8_t[:, :, 0:1]

            ti8 = sigpool.tile([128, DEPTH, 4, NI_SH], FP8, name="ti8")
            td8 = sigpool.tile([128, DEPTH, 4, ND_SH], FP8, name="td8")
            tisb = mpool.tile([128, DEPTH, I_CHUNKS], F32, name="tisb")
            parts = mpool.tile([128, N_STRIPES, I_CHUNKS], F32, name="parts")
            y_acc = mpool.tile([128, I_CHUNKS], F32, name="y")

            # ---- Phase 1: i-side gates (sigmoid -> fp8) + Si row-sums ----
            with (
                tc.tile_pool(name="sip", bufs=1, space="PSUM") as sipool,
                tc.tile_pool(name="g1", bufs=3, space="PSUM") as g1pool,
            ):
                si = sipool.tile([128, DEPTH, I_CHUNKS], F32)
                for l in range(DEPTH):
                    for nb in range(NB):
                        cols = slice(nb * 512, (nb + 1) * 512)
                        for mp in range(2):
                            _gate_pair(
                                nc, g1pool, "g", wq, xi, ti8, l, mp, cols,
                                AFT.Sigmoid, SIG_SCALE,
                            )
                    # Si for the PREVIOUS layer: keeps PE from stalling on
                    # this layer's activations.
                    if l > 0:
                        for ic in range(I_CHUNKS):
                            isl = slice(ic * 128, (ic + 1) * 128)
                            for h in range(2):
                                nc.tensor.matmul(
                                    si[:, l - 1, ic : ic + 1],
                                    ti8[:, l - 1, 2 * h : 2 * h + 2, isl],
                                    ones8,
                                    start=(h == 0),
                                    stop=(h == 1),
                                    perf_mode=DRM,
                                )
                for ic in range(I_CHUNKS):
                    isl = slice(ic * 128, (ic + 1) * 128)
                    for h in range(2):
                        nc.tensor.matmul(
                            si[:, DEPTH - 1, ic : ic + 1],
                            ti8[:, DEPTH - 1, 2 * h : 2 * h + 2, isl],
                            ones8,
                            start=(h == 0),
                            stop=(h == 1),
                            perf_mode=DRM,
                        )
                nc.vector.tensor_copy(tisb[:], si[:])

            # ---- Phase 2: d-side gates + K products + combine, by stripe ----
            with tc.tile_pool(name="kx", bufs=2, space="PSUM") as kxpool:
                for s in range(N_STRIPES):
                    scols = slice(s * D_STRIPE, (s + 1) * D_STRIPE)
                    for l in range(DEPTH):
                        for mp in range(2):
                            _gate_pair(
                                nc, kxpool, "AB", wq, xd, td8, l, mp, scols,
                                AFT.Tanh, TANH_SCALE,
                            )
                    for ic in range(I_CHUNKS):
                        isl = slice(ic * 128, (ic + 1) * 128)
                        kA = kxpool.tile([128, 2, 512], F32, tag="AB", name=f"kA{s}_{ic}")
                        kB = kxpool.tile([128, 2, 512], F32, tag="B", name=f"kB{s}_{ic}")
                        for l in range(DEPTH):
                            dst = kA if l < 2 else kB
                            slot = l % 2
                            for h in range(2):
                                nc.tensor.matmul(
                                    dst[:, slot, :],
                                    ti8[:, l, 2 * h : 2 * h + 2, isl],
                                    td8[:, l, 2 * h : 2 * h + 2, scols],
                                    start=(h == 0),
                                    stop=(h == 1),
                                    perf_mode=DRM,
                                )
                        kblk = kpool.tile([128, D_STRIPE], F32, tag="kblk")
                        nc.vector.scalar_tensor_tensor(
                            kblk[:], kA[:, 0, :], tisb[:, 0, ic : ic + 1],
                            alp[:, scols], ADD, MULT,
                        )
                        nc.vector.scalar_tensor_tensor(
                            kblk[:], kA[:, 1, :], tisb[:, 1, ic : ic + 1],
                            kblk[:], ADD, MULT,
                        )
                        u2 = upool.tile([128, D_STRIPE], F32, tag="u")
                        nc.scalar.activation(
                            u2[:], kB[:, 0, :], AFT.Identity,
                            bias=tisb[:, 2, ic : ic + 1],
                        )
                        nc.gpsimd.tensor_mul(kblk[:], u2[:], kblk[:])
                        if (s * I_CHUNKS + ic) % 8 < 3:
                            # direct PSUM read on DVE (balances ACT load)
                            nc.vector.scalar_tensor_tensor(
                                kblk[:], kB[:, 1, :], tisb[:, 3, ic : ic + 1],
                                kblk[:], ADD, MULT,
                                accum_out=parts[:, s, ic : ic + 1],
                            )
                        else:
                            u3 = upool.tile([128, D_STRIPE], F32, tag="u")
                            nc.scalar.activation(
                                u3[:], kB[:, 1, :], AFT.Identity,
                                bias=tisb[:, 3, ic : ic + 1],
                            )
                            nc.vector.scalar_tensor_tensor(
                                kblk[:], u3[:], 0.0,
                                kblk[:], ADD, MULT,
                                accum_out=parts[:, s, ic : ic + 1],
                            )

            nc.vector.tensor_add(y_acc[:], parts[:, 0], parts[:, 1])
            nc.vector.tensor_add(y_acc[:], y_acc[:], parts[:, 2])
            nc.vector.tensor_add(y_acc[:], y_acc[:], parts[:, 3])
            nc.sync.dma_start(y_d.ap(), y_acc[:])

    nc.compile()
    return nc


def _get_nc():
    global _NC
    if _NC is None:
        _NC = _build()
    return _NC


def _hilo(x):
    import ml_dtypes

    f8 = ml_dtypes.float8_e4m3
    hi = x.astype(f8)
    lo = (x - hi.astype(np.float32)).astype(f8)
    return hi, lo


def _dr_pack(mat):
    """[512, n] -> [128, 2(h), 2(j), n] with row w = h*256 + j*128 + p."""
    n = mat.shape[1]
    return np.ascontiguousarray(
        mat.reshape(2, 2, 128, n).transpose(2, 0, 1, 3)
    )


def make_in_maps(inp, data, gating, alphas):
    inp = np.asarray(inp, dtype=np.float32)
    data = np.asarray(data, dtype=np.float32)
    gating = np.asarray(gating, dtype=np.float64)
    alphas = np.asarray(alphas, dtype=np.float32) * np.float32(C_SCALE)

    # Cumulative weight products (flattened gate chain), fp8 hi/lo pairs
    import ml_dtypes

    wt = gating[0]
    wq = np.empty((128, DEPTH, 2, 2, 2, DIM), dtype=ml_dtypes.float8_e4m3)
    for l in range(DEPTH):
        if l > 0:
            wt = wt @ gating[l]
        wh, wl = _hilo((WSC * wt).astype(np.float32))
        wq[:, l, 0] = _dr_pack(wh)
        wq[:, l, 1] = _dr_pack(wl)

    def pack_x(xT):  # [512, cols] fp32 -> [128, 2, 2, 2, cols] fp8
        xh, xl = _hilo(XSC * xT)
        out = np.empty((128, 2, 2, 2, xT.shape[1]), dtype=ml_dtypes.float8_e4m3)
        out[:, 0] = _dr_pack(xh)
        out[:, 1] = _dr_pack(xl)
        return out

    in_maps = []
    for r in range(R):
        xi = pack_x(np.ascontiguousarray(inp[r * NI_SH : (r + 1) * NI_SH].T))
        for c in range(C):
            xd = pack_x(np.ascontiguousarray(data[c * ND_SH : (c + 1) * ND_SH].T))
            al = np.ascontiguousarray(
                np.broadcast_to(alphas[c * ND_SH : (c + 1) * ND_SH], (128, ND_SH))
            )
            in_maps.append({"wq": wq, "xi": xi, "xd": xd, "alphas_s": al})
    return in_maps


def kernel(inp, data, gating, alphas):
    nc = _get_nc()
    in_maps = make_in_maps(inp, data, gating, alphas)
    res = run_bass_kernel_spmd(nc, in_maps, core_ids=list(range(R * C))).results

    y = np.empty(N_I, dtype=np.float32)
    for r in range(R):
        acc = res[r * C]["y"].T.reshape(NI_SH).copy()
        for c in range(1, C):
            acc += res[r * C + c]["y"].T.reshape(NI_SH)
        y[r * NI_SH : (r + 1) * NI_SH] = acc
    return y


# revision 12
# speedup vs baseline: 1.0392x; 1.0392x over previous
"""Trainium2 Bass kernel for the DLGN kernel-machine problem.

Reference computation (fp32):
    ig = inp @ gating[0]; dg = data @ gating[0]
    K  = sig(B*ig) @ sig(B*dg).T
    for l in 1..3:
        ig = ig @ gating[l]; dg = dg @ gating[l]
        K *= (sig(B*ig) @ sig(B*dg).T) / 512
    out = K @ alphas                      # [n_inp]

Strategy (8 NeuronCores, 2x4 shard as before), v2 rewrite:
  - FLATTENED GATE CHAIN: host precomputes cumulative weight products
    W~_l = W_1...W_l (fp64), so layer l's pre-activation is x0 @ W~_l
    directly from the original input. This removes the sequential
    layer dependency and all PSUM->SBUF chain copies (was ~63us DVE).
  - fp8 DoubleRow gate matmuls with hi/lo split operands: x ~ xh + xl
    and W~ ~ Wh + Wl (e4m3 pairs, ~14-bit effective). Three-term
    product (xh@Wh + xl@Wh + xh@Wl, lo@lo dropped) keeps the final
    metric at ~1.5e-2 (measured in numpy emulation; gate is 2e-2)
    while running the gates at fp8-DR speed: 2x fewer PE cycles than
    bf16 even with 3 terms.
  - Asymmetric centering (as before): i-side sigmoid s, d-side
    t = tanh(2x); 2K_l = Si_l + s8.t8 with Si = rowsum(s8) EXACT via
    fp8 matmul vs ones. All matmuls fp8-DR: no dtype mode switches.
  - Combine, one fixed schedule per (stripe, ic) tile:
      DVE  stt: kblk = (kps0 + S0) * alphas      (PSUM read)
      DVE  stt: kblk = (kps1 + S1) * kblk        (PSUM read)
      ACT  copy: u23 = kps2|kps3 [128,2,512]     (paired 2-bank PSUM read)
      Pool stt: kblk = (u23[0] + S2) * kblk      (SBUF, gpsimd)
      DVE  stt: (u23[1] + S3) * kblk, accum_out -> parts[s,ic]
    Engine busy est/core: PE ~137us, ACT ~134, DVE ~118, Pool ~45.
  - PSUM: phase 1 (i-gates): pair pool [128,2,512]x3 + Si bank = 7.
    phase 2: shared pool tags AB (d-gates + K L0|L1) 2x2 banks and
    B (K L2|L3) 2x2 banks = 8.
"""

import numpy as np

import concourse.tile as tile
from concourse import bacc, mybir
from concourse.bass_utils import run_bass_kernel_spmd

BETA = 4.0
WIDTH = 512
DEPTH = 4
DIM = 512
N_I = 4096
N_D = 8192
R, C = 2, 4
NI_SH = N_I // R  # 2048
ND_SH = N_D // C  # 2048
D_STRIPE = 512
N_STRIPES = ND_SH // D_STRIPE  # 4
I_CHUNKS = NI_SH // 128  # 16
NB = NI_SH // 512  # 4 column blocks on the i side
C_SCALE = (0.5**DEPTH) / float(WIDTH ** (DEPTH - 1))  # 2^-31, exact
WSC = 32.0  # weight prescale for fp8
XSC = 16.0  # input prescale for fp8
SIG_SCALE = BETA / (WSC * XSC)  # sig(4x) = sig(psum * 2^-7)
TANH_SCALE = (BETA / 2) / (WSC * XSC)  # tanh(2x) = tanh(psum * 2^-8)

F32 = mybir.dt.float32
FP8 = mybir.dt.float8e4
AFT = mybir.ActivationFunctionType
MULT = mybir.AluOpType.mult
ADD = mybir.AluOpType.add
DRM = mybir.MatmulPerfMode.DoubleRow

_NC = None

# (x-variant, w-variant) matmul terms: hi@hi + lo@hi (+ hi@lo when W_HILO)
W_HILO = True
HILO_TERMS = [(0, 0), (1, 0)] + ([(0, 1)] if W_HILO else [])


def _gate_pair(nc, pool, tag, wq, x8, out8, l, mp, blk, aft, scale):
    """One [128, 2, 512] gate pair tile: fp8-DR matmuls + 1 activation.

    mp is the m-chunk pair (m = 2*mp + m2); blk is the 512-wide column
    block of the i/d axis being produced."""
    gt = pool.tile([128, 2, 512], F32, tag=tag)
    nterm = 2 * len(HILO_TERMS)
    for m2 in range(2):
        m = 2 * mp + m2
        n = 0
        for h in range(2):
            for xv, wv in HILO_TERMS:
                nc.tensor.matmul(
                    gt[:, m2, :],
                    wq[:, l, wv, h, :, m * 128 : (m + 1) * 128],
                    x8[:, blk, xv, h, :, :],
                    start=(n == 0),
                    stop=(n == nterm - 1),
                    perf_mode=DRM,
                )
                n += 1
    cols = slice(blk * 512, (blk + 1) * 512)
    nc.scalar.activation(out8[:, l, 2 * mp : 2 * mp + 2, cols], gt[:], aft, scale=scale)


def _build(repeat=1):
    nc = bacc.Bacc("TRN2", target_bir_lowering=False, debug=False, num_devices=8)

    wq_d = nc.dram_tensor("wq", [128, DEPTH, 2, 2, 2, DIM], FP8, kind="ExternalInput")
    xi_d = nc.dram_tensor("xi", [128, NB, 2, 2, 2, 512], FP8, kind="ExternalInput")
    xd_d = nc.dram_tensor("xd", [128, N_STRIPES, 2, 2, 2, 512], FP8, kind="ExternalInput")
    alphas_d = nc.dram_tensor("alphas_s", [128, ND_SH], F32, kind="ExternalInput")
    y_d = nc.dram_tensor("y", [128, I_CHUNKS], F32, kind="ExternalOutput")

    from contextlib import nullcontext

    with tile.TileContext(nc) as tc:
        with (
            tc.tile_pool(name="w", bufs=1) as wpool,
            tc.tile_pool(name="x", bufs=1) as xpool,
            tc.tile_pool(name="sig", bufs=1) as sigpool,
            tc.tile_pool(name="misc", bufs=1) as mpool,
            tc.tile_pool(name="kblk", bufs=4) as kpool,
            tc.tile_pool(name="u", bufs=6) as upool,
            tc.For_i(0, repeat, 1) if repeat > 1 else nullcontext(),
        ):
            wq = wpool.tile([128, DEPTH, 2, 2, 2, DIM], FP8)
            for l in range(DEPTH):
                nc.sync.dma_start(wq[:, l], wq_d.ap()[:, l])
            xi = xpool.tile([128, NB, 2, 2, 2, 512], FP8, name="xi")
            xd = xpool.tile([128, N_STRIPES, 2, 2, 2, 512], FP8, name="xd")
            alp = mpool.tile([128, ND_SH], F32, name="alp")
            nc.sync.dma_start(xi[:, 0], xi_d.ap()[:, 0])
            nc.sync.dma_start(xd[:, 0], xd_d.ap()[:, 0])
            nc.sync.dma_start(alp[:], alphas_d.ap())
            for nb in range(1, NB):
                nc.sync.dma_start(xi[:, nb], xi_d.ap()[:, nb])
            for s in range(1, N_STRIPES):
                nc.sync.dma_start(xd[:, s], xd_d.ap()[:, s])

            ones8_t = mpool.tile([128, 2, 16], FP8, name="ones8")
            nc.gpsimd.memset(ones8_t[:], 1.0)
            ones8 = ones8_t[:, :, 0:1]

            ti8 = sigpool.tile([128, DEPTH, 4, NI_SH], FP8, name="ti8")
            td8 = sigpool.tile([128, DEPTH, 4, ND_SH], FP8, name="td8")
            tisb = mpool.tile([128, DEPTH, I_CHUNKS], F32, name="tisb")
            parts = mpool.tile([128, N_STRIPES, I_CHUNKS], F32, name="parts")
            y_acc = mpool.tile([128, I_CHUNKS], F32, name="y")

            # ---- Unified pipeline: i-gates, d-gates, K products and combine
            # all interleave through one PSUM scope (AB pair ring 4 banks +
            # B single ring 3 banks + Si bank = 8). i-gates for column block
            # nb trickle through stripe 0's combine iterations just ahead of
            # their first use; stripe s+1's d-gates trickle through stripe
            # s. Each tile's combine tail (ACT u-copy, Pool mul, final stt)
            # is emitted one tile late so in-order engine queues don't
            # head-of-line block on the cross-engine dependency chain.
            with (
                tc.tile_pool(name="sip", bufs=1, space="PSUM") as sipool,
                tc.tile_pool(name="kx", bufs=2, space="PSUM") as kxpool,
            ):
                si = sipool.tile([128, DEPTH, I_CHUNKS], F32)

                def _ig(nb, k):
                    """Emit i-gate pair k (of 8) for column block nb."""
                    _gate_pair(
                        nc, kxpool, "AB", wq, xi, ti8, k // 2, k % 2,
                        nb, AFT.Sigmoid, SIG_SCALE,
                    )

                def _si_group(nb):
                    """Si row-sum matmuls + tisb copy for ic block nb."""
                    for ic in range(4 * nb, 4 * nb + 4):
                        isl = slice(ic * 128, (ic + 1) * 128)
                        for l in range(DEPTH):
                            for h in range(2):
                                nc.tensor.matmul(
                                    si[:, l, ic : ic + 1],
                                    ti8[:, l, 2 * h : 2 * h + 2, isl],
                                    ones8,
                                    start=(h == 0),
                                    stop=(h == 1),
                                    perf_mode=DRM,
                                )
                    cs = slice(4 * nb, 4 * nb + 4)
                    nc.vector.tensor_copy(tisb[:, :, cs], si[:, :, cs])

                def _dg(s, k):
                    """Emit d-gate pair k (of 8) for stripe s."""
                    _gate_pair(
                        nc, kxpool, "AB", wq, xd, td8, k // 2, k % 2,
                        s, AFT.Tanh, TANH_SCALE,
                    )

                pend = None  # (s, ic, kblk, kB2, kB3) awaiting the tail

                def _stage23(s, ic, kblk, kB2, kB3):
                    u2 = upool.tile([128, D_STRIPE], F32, tag="u", name=f"u2_{s}_{ic}")
                    nc.scalar.activation(
                        u2[:], kB2[:], AFT.Identity, bias=tisb[:, 2, ic : ic + 1]
                    )
                    nc.gpsimd.tensor_mul(kblk[:], u2[:], kblk[:])
                    if (s * I_CHUNKS + ic) % 8 < 3:
                        # direct PSUM read on DVE (balances ACT load)
                        nc.vector.scalar_tensor_tensor(
                            kblk[:], kB3[:], tisb[:, 3, ic : ic + 1],
                            kblk[:], ADD, MULT,
                            accum_out=parts[:, s, ic : ic + 1],
                        )
                    else:
                        u3 = upool.tile([128, D_STRIPE], F32, tag="u", name=f"u3_{s}_{ic}")
                        nc.scalar.activation(
                            u3[:], kB3[:], AFT.Identity, bias=tisb[:, 3, ic : ic + 1]
                        )
                        nc.vector.scalar_tensor_tensor(
                            kblk[:], u3[:], 0.0,
                            kblk[:], ADD, MULT,
                            accum_out=parts[:, s, ic : ic + 1],
                        )

                # prologue: i-gates nb=0 and d-gates s=0, interleaved
                for k in range(8):
                    _ig(0, k)
                    _dg(0, k)
                _si_group(0)

                # Diagonal wave order: tiles with s + nb == w run in wave w,
                # so each wave needs only gates finished in earlier waves and
                # the i/d gate load spreads evenly instead of piling onto
                # stripe 0.
                waves = []
                for w in range(N_STRIPES + NB - 1):
                    tl = []
                    for s in range(N_STRIPES):
                        nb = w - s
                        if 0 <= nb < NB:
                            tl.extend((s, 4 * nb + j) for j in range(4))
                    waves.append(tl)

                for w, tl in enumerate(waves):
                    nw = len(tl)
                    # gates needed by wave w+1, spread across this wave
                    gates = []
                    if w + 1 < NB:
                        gates += [("i", w + 1, k) for k in range(8)]
                    if w + 1 < N_STRIPES:
                        gates += [("d", w + 1, k) for k in range(8)]
                    for idx, (s, ic) in enumerate(tl):
                        g0 = (len(gates) * idx) // nw
                        g1 = (len(gates) * (idx + 1)) // nw
                        for kind, x, k in gates[g0:g1]:
                            (_ig if kind == "i" else _dg)(x, k)
                        if w + 1 < NB and idx + 1 == nw:
                            _si_group(w + 1)
                        scols = slice(s * D_STRIPE, (s + 1) * D_STRIPE)
                        isl = slice(ic * 128, (ic + 1) * 128)
                        kA = kxpool.tile(
                            [128, 2, 512], F32, tag="AB", name=f"kA{s}_{ic}"
                        )
                        kB2 = kxpool.tile(
                            [128, 512], F32, tag="B", name=f"kB2_{s}_{ic}"
                        )
                        kB3 = kxpool.tile(
                            [128, 512], F32, tag="B", name=f"kB3_{s}_{ic}"
                        )
                        for l in range(DEPTH):
                            dst = (kA[:, 0, :], kA[:, 1, :], kB2[:], kB3[:])[l]
                            for h in range(2):
                                nc.tensor.matmul(
                                    dst,
                                    ti8[:, l, 2 * h : 2 * h + 2, isl],
                                    td8[:, l, 2 * h : 2 * h + 2, scols],
                                    start=(h == 0),
                                    stop=(h == 1),
                                    perf_mode=DRM,
                                )
                        kblk = kpool.tile([128, D_STRIPE], F32, tag="kblk")
                        nc.vector.scalar_tensor_tensor(
                            kblk[:], kA[:, 0, :], tisb[:, 0, ic : ic + 1],
                            alp[:, scols], ADD, MULT,
                        )
                        nc.vector.scalar_tensor_tensor(
                            kblk[:], kA[:, 1, :], tisb[:, 1, ic : ic + 1],
                            kblk[:], ADD, MULT,
                        )
                        if pend is not None:
                            _stage23(*pend)
                        pend = (s, ic, kblk, kB2, kB3)
                _stage23(*pend)

            nc.vector.tensor_add(y_acc[:], parts[:, 0], parts[:, 1])
            nc.vector.tensor_add(y_acc[:], y_acc[:], parts[:, 2])
            nc.vector.tensor_add(y_acc[:], y_acc[:], parts[:, 3])
            nc.sync.dma_start(y_d.ap(), y_acc[:])

    nc.compile()
    return nc


def _get_nc():
    global _NC
    if _NC is None:
        _NC = _build()
    return _NC


def _hilo(x):
    import ml_dtypes

    f8 = ml_dtypes.float8_e4m3
    hi = x.astype(f8)
    lo = (x - hi.astype(np.float32)).astype(f8)
    return hi, lo


def _dr_pack(mat):
    """[512, n] -> [128, 2(h), 2(j), n] with row w = h*256 + j*128 + p."""
    n = mat.shape[1]
    return np.ascontiguousarray(
        mat.reshape(2, 2, 128, n).transpose(2, 0, 1, 3)
    )


def make_in_maps(inp, data, gating, alphas):
    inp = np.asarray(inp, dtype=np.float32)
    data = np.asarray(data, dtype=np.float32)
    gating = np.asarray(gating, dtype=np.float64)
    alphas = np.asarray(alphas, dtype=np.float32) * np.float32(C_SCALE)

    # Cumulative weight products (flattened gate chain), fp8 hi/lo pairs
    import ml_dtypes

    wt = gating[0]
    wq = np.empty((128, DEPTH, 2, 2, 2, DIM), dtype=ml_dtypes.float8_e4m3)
    for l in range(DEPTH):
        if l > 0:
            wt = wt @ gating[l]
        wh, wl = _hilo((WSC * wt).astype(np.float32))
        wq[:, l, 0] = _dr_pack(wh)
        wq[:, l, 1] = _dr_pack(wl)

    def pack_x(xT):  # [512, cols] fp32 -> [128, nblk, 2, 2, 2, 512] fp8
        nblk = xT.shape[1] // 512
        xh, xl = _hilo(XSC * xT)
        out = np.empty((128, nblk, 2, 2, 2, 512), dtype=ml_dtypes.float8_e4m3)
        for b in range(nblk):
            cs = slice(b * 512, (b + 1) * 512)
            out[:, b, 0] = _dr_pack(xh[:, cs])
            out[:, b, 1] = _dr_pack(xl[:, cs])
        return out

    in_maps = []
    for r in range(R):
        xi = pack_x(np.ascontiguousarray(inp[r * NI_SH : (r + 1) * NI_SH].T))
        for c in range(C):
            xd = pack_x(np.ascontiguousarray(data[c * ND_SH : (c + 1) * ND_SH].T))
            al = np.ascontiguousarray(
                np.broadcast_to(alphas[c * ND_SH : (c + 1) * ND_SH], (128, ND_SH))
            )
            in_maps.append({"wq": wq, "xi": xi, "xd": xd, "alphas_s": al})
    return in_maps


def kernel(inp, data, gating, alphas):
    nc = _get_nc()
    in_maps = make_in_maps(inp, data, gating, alphas)
    res = run_bass_kernel_spmd(nc, in_maps, core_ids=list(range(R * C))).results

    y = np.empty(N_I, dtype=np.float32)
    for r in range(R):
        acc = res[r * C]["y"].T.reshape(NI_SH).copy()
        for c in range(1, C):
            acc += res[r * C + c]["y"].T.reshape(NI_SH)
        y[r * NI_SH : (r + 1) * NI_SH] = acc
    return y


# revision 13
# speedup vs baseline: 1.1772x; 1.1327x over previous
"""Trainium2 Bass kernel for the DLGN kernel-machine problem.

Reference computation (fp32):
    ig = inp @ gating[0]; dg = data @ gating[0]
    K  = sig(B*ig) @ sig(B*dg).T
    for l in 1..3:
        ig = ig @ gating[l]; dg = dg @ gating[l]
        K *= (sig(B*ig) @ sig(B*dg).T) / 512
    out = K @ alphas                      # [n_inp]

Strategy (8 NeuronCores, 2x4 shard as before), v2 rewrite:
  - FLATTENED GATE CHAIN: host precomputes cumulative weight products
    W~_l = W_1...W_l (fp64), so layer l's pre-activation is x0 @ W~_l
    directly from the original input. This removes the sequential
    layer dependency and all PSUM->SBUF chain copies (was ~63us DVE).
  - fp8 DoubleRow gate matmuls with hi/lo split operands: x ~ xh + xl
    and W~ ~ Wh + Wl (e4m3 pairs, ~14-bit effective). Three-term
    product (xh@Wh + xl@Wh + xh@Wl, lo@lo dropped) keeps the final
    metric at ~1.5e-2 (measured in numpy emulation; gate is 2e-2)
    while running the gates at fp8-DR speed: 2x fewer PE cycles than
    bf16 even with 3 terms.
  - Asymmetric centering (as before): i-side sigmoid s, d-side
    t = tanh(2x); 2K_l = Si_l + s8.t8 with Si = rowsum(s8) EXACT via
    fp8 matmul vs ones. All matmuls fp8-DR: no dtype mode switches.
  - Combine, one fixed schedule per (stripe, ic) tile:
      DVE  stt: kblk = (kps0 + S0) * alphas      (PSUM read)
      DVE  stt: kblk = (kps1 + S1) * kblk        (PSUM read)
      ACT  copy: u23 = kps2|kps3 [128,2,512]     (paired 2-bank PSUM read)
      Pool stt: kblk = (u23[0] + S2) * kblk      (SBUF, gpsimd)
      DVE  stt: (u23[1] + S3) * kblk, accum_out -> parts[s,ic]
    Engine busy est/core: PE ~137us, ACT ~134, DVE ~118, Pool ~45.
  - PSUM: phase 1 (i-gates): pair pool [128,2,512]x3 + Si bank = 7.
    phase 2: shared pool tags AB (d-gates + K L0|L1) 2x2 banks and
    B (K L2|L3) 2x2 banks = 8.
"""

import numpy as np

import concourse.tile as tile
from concourse import bacc, mybir
from concourse.bass_utils import run_bass_kernel_spmd

BETA = 4.0
WIDTH = 512
DEPTH = 4
DIM = 512
N_I = 4096
N_D = 8192
R, C = 2, 4
NI_SH = N_I // R  # 2048
ND_SH = N_D // C  # 2048
D_STRIPE = 512
N_STRIPES = ND_SH // D_STRIPE  # 4
I_CHUNKS = NI_SH // 128  # 16
NB = NI_SH // 512  # 4 column blocks on the i side
C_SCALE = (0.5**DEPTH) / float(WIDTH ** (DEPTH - 1))  # 2^-31, exact
WSC = 32.0  # weight prescale for fp8
XSC = 16.0  # input prescale for fp8
SIG_SCALE = BETA / (WSC * XSC)  # sig(4x) = sig(psum * 2^-7)
TANH_SCALE = (BETA / 2) / (WSC * XSC)  # tanh(2x) = tanh(psum * 2^-8)

F32 = mybir.dt.float32
FP8 = mybir.dt.float8e4
AFT = mybir.ActivationFunctionType
MULT = mybir.AluOpType.mult
ADD = mybir.AluOpType.add
DRM = mybir.MatmulPerfMode.DoubleRow

_NC = None

# (x-variant, w-variant) matmul terms: hi@hi + lo@hi (+ hi@lo when W_HILO)
W_HILO = False
HILO_TERMS = [(0, 0), (1, 0)] + ([(0, 1)] if W_HILO else [])


def _gate_pair(nc, pool, tag, wq, x8, out8, l, mp, blk, aft, scale):
    """One [128, 2, 512] gate pair tile: fp8-DR matmuls + 1 activation.

    mp is the m-chunk pair (m = 2*mp + m2); blk is the 512-wide column
    block of the i/d axis being produced."""
    gt = pool.tile([128, 2, 512], F32, tag=tag)
    nterm = 2 * len(HILO_TERMS)
    for m2 in range(2):
        m = 2 * mp + m2
        n = 0
        for h in range(2):
            for xv, wv in HILO_TERMS:
                nc.tensor.matmul(
                    gt[:, m2, :],
                    wq[:, l, wv, h, :, m * 128 : (m + 1) * 128],
                    x8[:, blk, xv, h, :, :],
                    start=(n == 0),
                    stop=(n == nterm - 1),
                    perf_mode=DRM,
                )
                n += 1
    cols = slice(blk * 512, (blk + 1) * 512)
    nc.scalar.activation(out8[:, l, 2 * mp : 2 * mp + 2, cols], gt[:], aft, scale=scale)


def _build(repeat=1):
    nc = bacc.Bacc("TRN2", target_bir_lowering=False, debug=False, num_devices=8)

    wq_d = nc.dram_tensor("wq", [128, DEPTH, 2, 2, 2, DIM], FP8, kind="ExternalInput")
    xi_d = nc.dram_tensor("xi", [128, NB, 2, 2, 2, 512], FP8, kind="ExternalInput")
    xd_d = nc.dram_tensor("xd", [128, N_STRIPES, 2, 2, 2, 512], FP8, kind="ExternalInput")
    alphas_d = nc.dram_tensor("alphas_s", [128, ND_SH], F32, kind="ExternalInput")
    y_d = nc.dram_tensor("y", [128, I_CHUNKS], F32, kind="ExternalOutput")

    from contextlib import nullcontext

    with tile.TileContext(nc) as tc:
        with (
            tc.tile_pool(name="w", bufs=1) as wpool,
            tc.tile_pool(name="x", bufs=1) as xpool,
            tc.tile_pool(name="sig", bufs=1) as sigpool,
            tc.tile_pool(name="misc", bufs=1) as mpool,
            tc.tile_pool(name="kblk", bufs=4) as kpool,
            tc.tile_pool(name="u", bufs=6) as upool,
            tc.For_i(0, repeat, 1) if repeat > 1 else nullcontext(),
        ):
            wq = wpool.tile([128, DEPTH, 2, 2, 2, DIM], FP8)
            for l in range(DEPTH):
                nc.sync.dma_start(wq[:, l], wq_d.ap()[:, l])
            xi = xpool.tile([128, NB, 2, 2, 2, 512], FP8, name="xi")
            xd = xpool.tile([128, N_STRIPES, 2, 2, 2, 512], FP8, name="xd")
            alp = mpool.tile([128, ND_SH], F32, name="alp")
            nc.sync.dma_start(xi[:, 0], xi_d.ap()[:, 0])
            nc.sync.dma_start(xd[:, 0], xd_d.ap()[:, 0])
            nc.sync.dma_start(alp[:], alphas_d.ap())
            for nb in range(1, NB):
                nc.sync.dma_start(xi[:, nb], xi_d.ap()[:, nb])
            for s in range(1, N_STRIPES):
                nc.sync.dma_start(xd[:, s], xd_d.ap()[:, s])

            ones8_t = mpool.tile([128, 2, 16], FP8, name="ones8")
            nc.gpsimd.memset(ones8_t[:], 1.0)
            ones8 = ones8_t[:, :, 0:1]

            ti8 = sigpool.tile([128, DEPTH, 4, NI_SH], FP8, name="ti8")
            td8 = sigpool.tile([128, DEPTH, 4, ND_SH], FP8, name="td8")
            tisb = mpool.tile([128, DEPTH, I_CHUNKS], F32, name="tisb")
            parts = mpool.tile([128, N_STRIPES, I_CHUNKS], F32, name="parts")
            y_acc = mpool.tile([128, I_CHUNKS], F32, name="y")

            # ---- Unified pipeline: i-gates, d-gates, K products and combine
            # all interleave through one PSUM scope (AB pair ring 4 banks +
            # B single ring 3 banks + Si bank = 8). i-gates for column block
            # nb trickle through stripe 0's combine iterations just ahead of
            # their first use; stripe s+1's d-gates trickle through stripe
            # s. Each tile's combine tail (ACT u-copy, Pool mul, final stt)
            # is emitted one tile late so in-order engine queues don't
            # head-of-line block on the cross-engine dependency chain.
            with (
                tc.tile_pool(name="sip", bufs=1, space="PSUM") as sipool,
                tc.tile_pool(name="kx", bufs=2, space="PSUM") as kxpool,
            ):
                si = sipool.tile([128, DEPTH, I_CHUNKS], F32)

                def _ig(nb, k):
                    """Emit i-gate pair k (of 8) for column block nb."""
                    _gate_pair(
                        nc, kxpool, "AB", wq, xi, ti8, k // 2, k % 2,
                        nb, AFT.Sigmoid, SIG_SCALE,
                    )

                def _si_group(nb):
                    """Si row-sum matmuls + tisb copy for ic block nb."""
                    for ic in range(4 * nb, 4 * nb + 4):
                        isl = slice(ic * 128, (ic + 1) * 128)
                        for l in range(DEPTH):
                            for h in range(2):
                                nc.tensor.matmul(
                                    si[:, l, ic : ic + 1],
                                    ti8[:, l, 2 * h : 2 * h + 2, isl],
                                    ones8,
                                    start=(h == 0),
                                    stop=(h == 1),
                                    perf_mode=DRM,
                                )
                    cs = slice(4 * nb, 4 * nb + 4)
                    nc.vector.tensor_copy(tisb[:, :, cs], si[:, :, cs])

                def _dg(s, k):
                    """Emit d-gate pair k (of 8) for stripe s."""
                    _gate_pair(
                        nc, kxpool, "AB", wq, xd, td8, k // 2, k % 2,
                        s, AFT.Tanh, TANH_SCALE,
                    )

                pend = None  # (s, ic, kblk, kB2, kB3) awaiting the tail

                def _stage23(s, ic, kblk, kB2, kB3):
                    u2 = upool.tile([128, D_STRIPE], F32, tag="u", name=f"u2_{s}_{ic}")
                    nc.scalar.activation(
                        u2[:], kB2[:], AFT.Identity, bias=tisb[:, 2, ic : ic + 1]
                    )
                    nc.gpsimd.tensor_mul(kblk[:], u2[:], kblk[:])
                    if (s * I_CHUNKS + ic) % 8 < 3:
                        # direct PSUM read on DVE (balances ACT load)
                        nc.vector.scalar_tensor_tensor(
                            kblk[:], kB3[:], tisb[:, 3, ic : ic + 1],
                            kblk[:], ADD, MULT,
                            accum_out=parts[:, s, ic : ic + 1],
                        )
                    else:
                        u3 = upool.tile([128, D_STRIPE], F32, tag="u", name=f"u3_{s}_{ic}")
                        nc.scalar.activation(
                            u3[:], kB3[:], AFT.Identity, bias=tisb[:, 3, ic : ic + 1]
                        )
                        nc.vector.scalar_tensor_tensor(
                            kblk[:], u3[:], 0.0,
                            kblk[:], ADD, MULT,
                            accum_out=parts[:, s, ic : ic + 1],
                        )

                # prologue: i-gates nb=0 and d-gates s=0, interleaved
                for k in range(8):
                    _ig(0, k)
                    _dg(0, k)
                _si_group(0)

                # Diagonal wave order: tiles with s + nb == w run in wave w,
                # so each wave needs only gates finished in earlier waves and
                # the i/d gate load spreads evenly instead of piling onto
                # stripe 0.
                waves = []
                for w in range(N_STRIPES + NB - 1):
                    tl = []
                    for s in range(N_STRIPES):
                        nb = w - s
                        if 0 <= nb < NB:
                            tl.extend((s, 4 * nb + j) for j in range(4))
                    waves.append(tl)

                for w, tl in enumerate(waves):
                    nw = len(tl)
                    # gates needed by wave w+1, spread across this wave
                    gates = []
                    if w + 1 < NB:
                        gates += [("i", w + 1, k) for k in range(8)]
                    if w + 1 < N_STRIPES:
                        gates += [("d", w + 1, k) for k in range(8)]
                    for idx, (s, ic) in enumerate(tl):
                        g0 = (len(gates) * idx) // nw
                        g1 = (len(gates) * (idx + 1)) // nw
                        for kind, x, k in gates[g0:g1]:
                            (_ig if kind == "i" else _dg)(x, k)
                        if w + 1 < NB and idx + 1 == nw:
                            _si_group(w + 1)
                        scols = slice(s * D_STRIPE, (s + 1) * D_STRIPE)
                        isl = slice(ic * 128, (ic + 1) * 128)
                        kA = kxpool.tile(
                            [128, 2, 512], F32, tag="AB", name=f"kA{s}_{ic}"
                        )
                        kB2 = kxpool.tile(
                            [128, 512], F32, tag="B", name=f"kB2_{s}_{ic}"
                        )
                        kB3 = kxpool.tile(
                            [128, 512], F32, tag="B", name=f"kB3_{s}_{ic}"
                        )
                        for l in range(DEPTH):
                            dst = (kA[:, 0, :], kA[:, 1, :], kB2[:], kB3[:])[l]
                            for h in range(2):
                                nc.tensor.matmul(
                                    dst,
                                    ti8[:, l, 2 * h : 2 * h + 2, isl],
                                    td8[:, l, 2 * h : 2 * h + 2, scols],
                                    start=(h == 0),
                                    stop=(h == 1),
                                    perf_mode=DRM,
                                )
                        kblk = kpool.tile([128, D_STRIPE], F32, tag="kblk")
                        nc.vector.scalar_tensor_tensor(
                            kblk[:], kA[:, 0, :], tisb[:, 0, ic : ic + 1],
                            alp[:, scols], ADD, MULT,
                        )
                        nc.vector.scalar_tensor_tensor(
                            kblk[:], kA[:, 1, :], tisb[:, 1, ic : ic + 1],
                            kblk[:], ADD, MULT,
                        )
                        if pend is not None:
                            _stage23(*pend)
                        pend = (s, ic, kblk, kB2, kB3)
                _stage23(*pend)

            nc.vector.tensor_add(y_acc[:], parts[:, 0], parts[:, 1])
            nc.vector.tensor_add(y_acc[:], y_acc[:], parts[:, 2])
            nc.vector.tensor_add(y_acc[:], y_acc[:], parts[:, 3])
            nc.sync.dma_start(y_d.ap(), y_acc[:])

    nc.compile()
    return nc


def _get_nc():
    global _NC
    if _NC is None:
        _NC = _build()
    return _NC


def _hilo(x):
    import ml_dtypes

    f8 = ml_dtypes.float8_e4m3
    hi = x.astype(f8)
    lo = (x - hi.astype(np.float32)).astype(f8)
    return hi, lo


def _dr_pack(mat):
    """[512, n] -> [128, 2(h), 2(j), n] with row w = h*256 + j*128 + p."""
    n = mat.shape[1]
    return np.ascontiguousarray(
        mat.reshape(2, 2, 128, n).transpose(2, 0, 1, 3)
    )


def make_in_maps(inp, data, gating, alphas):
    inp = np.asarray(inp, dtype=np.float32)
    data = np.asarray(data, dtype=np.float32)
    gating = np.asarray(gating, dtype=np.float64)
    alphas = np.asarray(alphas, dtype=np.float32) * np.float32(C_SCALE)

    # Cumulative weight products (flattened gate chain), fp8 hi/lo pairs
    import ml_dtypes

    wt = gating[0]
    wq = np.empty((128, DEPTH, 2, 2, 2, DIM), dtype=ml_dtypes.float8_e4m3)
    for l in range(DEPTH):
        if l > 0:
            wt = wt @ gating[l]
        wh, wl = _hilo((WSC * wt).astype(np.float32))
        wq[:, l, 0] = _dr_pack(wh)
        wq[:, l, 1] = _dr_pack(wl)

    def pack_x(xT):  # [512, cols] fp32 -> [128, nblk, 2, 2, 2, 512] fp8
        nblk = xT.shape[1] // 512
        xh, xl = _hilo(XSC * xT)
        out = np.empty((128, nblk, 2, 2, 2, 512), dtype=ml_dtypes.float8_e4m3)
        for b in range(nblk):
            cs = slice(b * 512, (b + 1) * 512)
            out[:, b, 0] = _dr_pack(xh[:, cs])
            out[:, b, 1] = _dr_pack(xl[:, cs])
        return out

    in_maps = []
    for r in range(R):
        xi = pack_x(np.ascontiguousarray(inp[r * NI_SH : (r + 1) * NI_SH].T))
        for c in range(C):
            xd = pack_x(np.ascontiguousarray(data[c * ND_SH : (c + 1) * ND_SH].T))
            al = np.ascontiguousarray(
                np.broadcast_to(alphas[c * ND_SH : (c + 1) * ND_SH], (128, ND_SH))
            )
            in_maps.append({"wq": wq, "xi": xi, "xd": xd, "alphas_s": al})
    return in_maps


def kernel(inp, data, gating, alphas):
    nc = _get_nc()
    in_maps = make_in_maps(inp, data, gating, alphas)
    res = run_bass_kernel_spmd(nc, in_maps, core_ids=list(range(R * C))).results

    y = np.empty(N_I, dtype=np.float32)
    for r in range(R):
        acc = res[r * C]["y"].T.reshape(NI_SH).copy()
        for c in range(1, C):
            acc += res[r * C + c]["y"].T.reshape(NI_SH)
        y[r * NI_SH : (r + 1) * NI_SH] = acc
    return y


# revision 14
# speedup vs baseline: 1.3011x; 1.1053x over previous
"""Trainium2 Bass kernel for the DLGN kernel-machine problem.

Reference computation (fp32):
    ig = inp @ gating[0]; dg = data @ gating[0]
    K  = sig(B*ig) @ sig(B*dg).T
    for l in 1..3:
        ig = ig @ gating[l]; dg = dg @ gating[l]
        K *= (sig(B*ig) @ sig(B*dg).T) / 512
    out = K @ alphas                      # [n_inp]

Strategy (8 NeuronCores, 2x4 shard as before), v2 rewrite:
  - FLATTENED GATE CHAIN: host precomputes cumulative weight products
    W~_l = W_1...W_l (fp64), so layer l's pre-activation is x0 @ W~_l
    directly from the original input. This removes the sequential
    layer dependency and all PSUM->SBUF chain copies (was ~63us DVE).
  - fp8 DoubleRow gate matmuls with hi/lo split operands: x ~ xh + xl
    and W~ ~ Wh + Wl (e4m3 pairs, ~14-bit effective). Three-term
    product (xh@Wh + xl@Wh + xh@Wl, lo@lo dropped) keeps the final
    metric at ~1.5e-2 (measured in numpy emulation; gate is 2e-2)
    while running the gates at fp8-DR speed: 2x fewer PE cycles than
    bf16 even with 3 terms.
  - Asymmetric centering (as before): i-side sigmoid s, d-side
    t = tanh(2x); 2K_l = Si_l + s8.t8 with Si = rowsum(s8) EXACT via
    fp8 matmul vs ones. All matmuls fp8-DR: no dtype mode switches.
  - Combine, one fixed schedule per (stripe, ic) tile:
      DVE  stt: kblk = (kps0 + S0) * alphas      (PSUM read)
      DVE  stt: kblk = (kps1 + S1) * kblk        (PSUM read)
      ACT  copy: u23 = kps2|kps3 [128,2,512]     (paired 2-bank PSUM read)
      Pool stt: kblk = (u23[0] + S2) * kblk      (SBUF, gpsimd)
      DVE  stt: (u23[1] + S3) * kblk, accum_out -> parts[s,ic]
    Engine busy est/core: PE ~137us, ACT ~134, DVE ~118, Pool ~45.
  - PSUM: phase 1 (i-gates): pair pool [128,2,512]x3 + Si bank = 7.
    phase 2: shared pool tags AB (d-gates + K L0|L1) 2x2 banks and
    B (K L2|L3) 2x2 banks = 8.
"""

import numpy as np

import concourse.tile as tile
from concourse import bacc, mybir
from concourse.bass_utils import run_bass_kernel_spmd

BETA = 4.0
WIDTH = 512
DEPTH = 4
DIM = 512
N_I = 4096
N_D = 8192
R, C = 2, 4
NI_SH = N_I // R  # 2048
ND_SH = N_D // C  # 2048
D_STRIPE = 512
N_STRIPES = ND_SH // D_STRIPE  # 4
I_CHUNKS = NI_SH // 128  # 16
NB = NI_SH // 512  # 4 column blocks on the i side
C_SCALE = (0.5**DEPTH) / float(WIDTH ** (DEPTH - 1))  # 2^-31, exact
WSC = 32.0  # weight prescale for fp8
XSC = 16.0  # input prescale for fp8
SIG_SCALE = BETA / (WSC * XSC)  # sig(4x) = sig(psum * 2^-7)
TANH_SCALE = (BETA / 2) / (WSC * XSC)  # tanh(2x) = tanh(psum * 2^-8)

F32 = mybir.dt.float32
FP8 = mybir.dt.float8e4
AFT = mybir.ActivationFunctionType
MULT = mybir.AluOpType.mult
ADD = mybir.AluOpType.add
DRM = mybir.MatmulPerfMode.DoubleRow

_NC = None

# (x-variant, w-variant) matmul terms: hi@hi + lo@hi (+ hi@lo when W_HILO)
W_HILO = False
HILO_TERMS = [(0, 0), (1, 0)] + ([(0, 1)] if W_HILO else [])


def _gate_pair(nc, pool, tag, wq, x8, out8, l, mp, blk, aft, scale):
    """One [128, 2, 512] gate pair tile: fp8-DR matmuls + 1 activation.

    mp is the m-chunk pair (m = 2*mp + m2); blk is the 512-wide column
    block of the i/d axis being produced."""
    gt = pool.tile([128, 2, 512], F32, tag=tag)
    nterm = 2 * len(HILO_TERMS)
    for m2 in range(2):
        m = 2 * mp + m2
        n = 0
        for h in range(2):
            for xv, wv in HILO_TERMS:
                nc.tensor.matmul(
                    gt[:, m2, :],
                    wq[:, l, wv, h, :, m * 128 : (m + 1) * 128],
                    x8[:, blk, xv, h, :, :],
                    start=(n == 0),
                    stop=(n == nterm - 1),
                    perf_mode=DRM,
                )
                n += 1
    cols = slice(blk * 512, (blk + 1) * 512)
    nc.scalar.activation(out8[:, l, 2 * mp : 2 * mp + 2, cols], gt[:], aft, scale=scale)


def _build(repeat=1):
    nc = bacc.Bacc("TRN2", target_bir_lowering=False, debug=False, num_devices=8)

    wq_d = nc.dram_tensor("wq", [128, DEPTH, 2, 2, 2, DIM], FP8, kind="ExternalInput")
    xi_d = nc.dram_tensor("xi", [128, NB, 2, 2, 2, 512], FP8, kind="ExternalInput")
    xd_d = nc.dram_tensor("xd", [128, N_STRIPES, 2, 2, 2, 512], FP8, kind="ExternalInput")
    alphas_d = nc.dram_tensor("alphas_s", [128, ND_SH], F32, kind="ExternalInput")
    y_d = nc.dram_tensor("y", [128, I_CHUNKS], F32, kind="ExternalOutput")

    from contextlib import nullcontext

    with tile.TileContext(nc) as tc:
        with (
            tc.tile_pool(name="w", bufs=1) as wpool,
            tc.tile_pool(name="x", bufs=1) as xpool,
            tc.tile_pool(name="sig", bufs=1) as sigpool,
            tc.tile_pool(name="misc", bufs=1) as mpool,
            tc.tile_pool(name="kblk", bufs=4) as kpool,
            tc.tile_pool(name="u", bufs=6) as upool,
            tc.For_i(0, repeat, 1) if repeat > 1 else nullcontext(),
        ):
            wq = wpool.tile([128, DEPTH, 2, 2, 2, DIM], FP8)
            for l in range(DEPTH):
                nc.sync.dma_start(wq[:, l], wq_d.ap()[:, l])
            xi = xpool.tile([128, NB, 2, 2, 2, 512], FP8, name="xi")
            xd = xpool.tile([128, N_STRIPES, 2, 2, 2, 512], FP8, name="xd")
            alp = mpool.tile([128, ND_SH], F32, name="alp")
            nc.sync.dma_start(xi[:, 0], xi_d.ap()[:, 0])
            nc.sync.dma_start(xd[:, 0], xd_d.ap()[:, 0])
            nc.sync.dma_start(alp[:], alphas_d.ap())
            for nb in range(1, NB):
                nc.sync.dma_start(xi[:, nb], xi_d.ap()[:, nb])
            for s in range(1, N_STRIPES):
                nc.sync.dma_start(xd[:, s], xd_d.ap()[:, s])

            ones8_t = mpool.tile([128, 2, 16], FP8, name="ones8")
            nc.gpsimd.memset(ones8_t[:], 1.0)
            ones8 = ones8_t[:, :, 0:1]

            ti8 = sigpool.tile([128, DEPTH, 4, NI_SH], FP8, name="ti8")
            td8 = sigpool.tile([128, DEPTH, 4, ND_SH], FP8, name="td8")
            tisb = mpool.tile([128, DEPTH, I_CHUNKS], F32, name="tisb")
            parts = mpool.tile([128, N_STRIPES, I_CHUNKS], F32, name="parts")
            y_acc = mpool.tile([128, I_CHUNKS], F32, name="y")

            # ---- Unified pipeline: i-gates, d-gates, K products and combine
            # all interleave through one PSUM scope (AB pair ring 4 banks +
            # B single ring 3 banks + Si bank = 8). i-gates for column block
            # nb trickle through stripe 0's combine iterations just ahead of
            # their first use; stripe s+1's d-gates trickle through stripe
            # s. Each tile's combine tail (ACT u-copy, Pool mul, final stt)
            # is emitted one tile late so in-order engine queues don't
            # head-of-line block on the cross-engine dependency chain.
            with (
                tc.tile_pool(name="sip", bufs=1, space="PSUM") as sipool,
                tc.tile_pool(name="kx", bufs=2, space="PSUM") as kxpool,
            ):
                si = sipool.tile([128, DEPTH, I_CHUNKS], F32)

                def _ig(nb, k):
                    """Emit i-gate pair k (of 8) for column block nb."""
                    _gate_pair(
                        nc, kxpool, "AB", wq, xi, ti8, k // 2, k % 2,
                        nb, AFT.Sigmoid, SIG_SCALE,
                    )

                def _si_group(nb):
                    """Si row-sum matmuls + tisb copy for ic block nb."""
                    for ic in range(4 * nb, 4 * nb + 4):
                        isl = slice(ic * 128, (ic + 1) * 128)
                        for l in range(DEPTH):
                            for h in range(2):
                                nc.tensor.matmul(
                                    si[:, l, ic : ic + 1],
                                    ti8[:, l, 2 * h : 2 * h + 2, isl],
                                    ones8,
                                    start=(h == 0),
                                    stop=(h == 1),
                                    perf_mode=DRM,
                                )
                    cs = slice(4 * nb, 4 * nb + 4)
                    nc.vector.tensor_copy(tisb[:, :, cs], si[:, :, cs])

                def _dg(s, k):
                    """Emit d-gate pair k (of 8) for stripe s."""
                    _gate_pair(
                        nc, kxpool, "AB", wq, xd, td8, k // 2, k % 2,
                        s, AFT.Tanh, TANH_SCALE,
                    )

                from collections import deque
                pend = deque()  # (s, ic, kblk, kB2, kB3) awaiting the tail
                LAG = 2

                def _stage23(s, ic, kblk, kB2, kB3):
                    u2 = upool.tile([128, D_STRIPE], F32, tag="u", name=f"u2_{s}_{ic}")
                    nc.scalar.activation(
                        u2[:], kB2[:], AFT.Identity, bias=tisb[:, 2, ic : ic + 1]
                    )
                    nc.gpsimd.tensor_mul(kblk[:], u2[:], kblk[:])
                    if (s * I_CHUNKS + ic) % 8 < 3:
                        # direct PSUM read on DVE (balances ACT load)
                        nc.vector.scalar_tensor_tensor(
                            kblk[:], kB3[:], tisb[:, 3, ic : ic + 1],
                            kblk[:], ADD, MULT,
                            accum_out=parts[:, s, ic : ic + 1],
                        )
                    else:
                        u3 = upool.tile([128, D_STRIPE], F32, tag="u", name=f"u3_{s}_{ic}")
                        nc.scalar.activation(
                            u3[:], kB3[:], AFT.Identity, bias=tisb[:, 3, ic : ic + 1]
                        )
                        nc.vector.scalar_tensor_tensor(
                            kblk[:], u3[:], 0.0,
                            kblk[:], ADD, MULT,
                            accum_out=parts[:, s, ic : ic + 1],
                        )

                # prologue: i-gates nb=0 and d-gates s=0, interleaved
                for k in range(8):
                    _ig(0, k)
                    _dg(0, k)
                _si_group(0)

                # Diagonal wave order: tiles with s + nb == w run in wave w,
                # so each wave needs only gates finished in earlier waves and
                # the i/d gate load spreads evenly instead of piling onto
                # stripe 0.
                waves = []
                for w in range(N_STRIPES + NB - 1):
                    tl = []
                    for s in range(N_STRIPES):
                        nb = w - s
                        if 0 <= nb < NB:
                            tl.extend((s, 4 * nb + j) for j in range(4))
                    waves.append(tl)

                for w, tl in enumerate(waves):
                    nw = len(tl)
                    # gates needed by wave w+1, spread across this wave
                    gates = []
                    if w + 1 < NB:
                        gates += [("i", w + 1, k) for k in range(8)]
                    if w + 1 < N_STRIPES:
                        gates += [("d", w + 1, k) for k in range(8)]
                    for idx, (s, ic) in enumerate(tl):
                        g0 = (len(gates) * idx) // nw
                        g1 = (len(gates) * (idx + 1)) // nw
                        for kind, x, k in gates[g0:g1]:
                            (_ig if kind == "i" else _dg)(x, k)
                        if w + 1 < NB and idx + 1 == nw:
                            _si_group(w + 1)
                        scols = slice(s * D_STRIPE, (s + 1) * D_STRIPE)
                        isl = slice(ic * 128, (ic + 1) * 128)
                        kA = kxpool.tile(
                            [128, 2, 512], F32, tag="AB", name=f"kA{s}_{ic}"
                        )
                        kB2 = kxpool.tile(
                            [128, 512], F32, tag="B", name=f"kB2_{s}_{ic}"
                        )
                        kB3 = kxpool.tile(
                            [128, 512], F32, tag="B", name=f"kB3_{s}_{ic}"
                        )
                        for l in range(DEPTH):
                            dst = (kA[:, 0, :], kA[:, 1, :], kB2[:], kB3[:])[l]
                            for h in range(2):
                                nc.tensor.matmul(
                                    dst,
                                    ti8[:, l, 2 * h : 2 * h + 2, isl],
                                    td8[:, l, 2 * h : 2 * h + 2, scols],
                                    start=(h == 0),
                                    stop=(h == 1),
                                    perf_mode=DRM,
                                )
                        kblk = kpool.tile([128, D_STRIPE], F32, tag="kblk")
                        nc.vector.scalar_tensor_tensor(
                            kblk[:], kA[:, 0, :], tisb[:, 0, ic : ic + 1],
                            alp[:, scols], ADD, MULT,
                        )
                        nc.vector.scalar_tensor_tensor(
                            kblk[:], kA[:, 1, :], tisb[:, 1, ic : ic + 1],
                            kblk[:], ADD, MULT,
                        )
                        pend.append((s, ic, kblk, kB2, kB3))
                        if len(pend) > LAG:
                            _stage23(*pend.popleft())
                while pend:
                    _stage23(*pend.popleft())

            nc.vector.tensor_add(y_acc[:], parts[:, 0], parts[:, 1])
            nc.vector.tensor_add(y_acc[:], y_acc[:], parts[:, 2])
            nc.vector.tensor_add(y_acc[:], y_acc[:], parts[:, 3])
            nc.sync.dma_start(y_d.ap(), y_acc[:])

    nc.compile()
    return nc


def _get_nc():
    global _NC
    if _NC is None:
        _NC = _build()
    return _NC


def _hilo(x):
    import ml_dtypes

    f8 = ml_dtypes.float8_e4m3
    hi = x.astype(f8)
    lo = (x - hi.astype(np.float32)).astype(f8)
    return hi, lo


def _dr_pack(mat):
    """[512, n] -> [128, 2(h), 2(j), n] with row w = h*256 + j*128 + p."""
    n = mat.shape[1]
    return np.ascontiguousarray(
        mat.reshape(2, 2, 128, n).transpose(2, 0, 1, 3)
    )


def make_in_maps(inp, data, gating, alphas):
    inp = np.asarray(inp, dtype=np.float32)
    data = np.asarray(data, dtype=np.float32)
    gating = np.asarray(gating, dtype=np.float64)
    alphas = np.asarray(alphas, dtype=np.float32) * np.float32(C_SCALE)

    # Cumulative weight products (flattened gate chain), fp8 hi/lo pairs
    import ml_dtypes

    wt = gating[0]
    wq = np.empty((128, DEPTH, 2, 2, 2, DIM), dtype=ml_dtypes.float8_e4m3)
    for l in range(DEPTH):
        if l > 0:
            wt = wt @ gating[l]
        wh, wl = _hilo((WSC * wt).astype(np.float32))
        wq[:, l, 0] = _dr_pack(wh)
        wq[:, l, 1] = _dr_pack(wl)

    def pack_x(xT):  # [512, cols] fp32 -> [128, nblk, 2, 2, 2, 512] fp8
        nblk = xT.shape[1] // 512
        xh, xl = _hilo(XSC * xT)
        out = np.empty((128, nblk, 2, 2, 2, 512), dtype=ml_dtypes.float8_e4m3)
        for b in range(nblk):
            cs = slice(b * 512, (b + 1) * 512)
            out[:, b, 0] = _dr_pack(xh[:, cs])
            out[:, b, 1] = _dr_pack(xl[:, cs])
        return out

    in_maps = []
    for r in range(R):
        xi = pack_x(np.ascontiguousarray(inp[r * NI_SH : (r + 1) * NI_SH].T))
        for c in range(C):
            xd = pack_x(np.ascontiguousarray(data[c * ND_SH : (c + 1) * ND_SH].T))
            al = np.ascontiguousarray(
                np.broadcast_to(alphas[c * ND_SH : (c + 1) * ND_SH], (128, ND_SH))
            )
            in_maps.append({"wq": wq, "xi": xi, "xd": xd, "alphas_s": al})
    return in_maps


def kernel(inp, data, gating, alphas):
    nc = _get_nc()
    in_maps = make_in_maps(inp, data, gating, alphas)
    res = run_bass_kernel_spmd(nc, in_maps, core_ids=list(range(R * C))).results

    y = np.empty(N_I, dtype=np.float32)
    for r in range(R):
        acc = res[r * C]["y"].T.reshape(NI_SH).copy()
        for c in range(1, C):
            acc += res[r * C + c]["y"].T.reshape(NI_SH)
        y[r * NI_SH : (r + 1) * NI_SH] = acc
    return y
